# revision 5
# baseline (speedup 1.0000x reference)
"""3-layer GAT (GATConv x3 + log_softmax) on 8 trn2 NeuronCores.

Strategy: 1-D node partition (6250 nodes/core). Edges live on the core that
owns their destination node, sorted by dst and packed into fixed-size
"windows" (<=128 dst nodes, K edge-tiles of 128 edges). Segment softmax +
scatter-add are done with per-tile one-hot selection matmuls accumulating in
PSUM. Node features for layer l+1 are produced per-window (fused projection)
and exchanged with an AllGather collective; layer-1 features are computed
replicated (Fin=12, cheap) so only two exchanges are needed.
"""
import numpy as np
import ml_dtypes

import concourse.bass as bass
import concourse.mybir as mybir
import concourse.tile as tile
from concourse.bass_utils import run_bass_kernel_spmd

BF = ml_dtypes.bfloat16
N = 50000
NC = 8
SHARD = N // NC            # 6250
H, C = 8, 64
F = H * C                  # 512
C3 = 5
F3 = H * C3                # 40
K_TILES = 8                # edge tiles per window (128 edges each)
G = 4                      # tiles per inner group
PAD_NODE = SHARD  # trash row
NEG_SLOPE = 0.2
DT_BF = mybir.dt.bfloat16
DT_F32 = mybir.dt.float32
DT_I32 = mybir.dt.int32
AF = mybir.ActivationFunctionType
ALU = mybir.AluOpType


def _split_drain_waits(nc, max_waits=1):
    # walrus on this toolchain rejects instructions carrying more than a few
    # sync waits; keep <=max_waits per instruction, move extras onto NoOps
    # inserted just before (same engine -> executes first, semantics kept).
    ctr = 0
    for f in nc.m.functions:
        for blk in f.blocks:
            new_list = []
            for ins in blk.instructions:
                if ins.sync_info and \
                        len(ins.sync_info.on_wait) > max_waits:
                    waits = list(ins.sync_info.on_wait)
                    keep, extra = waits[:max_waits], waits[max_waits:]
                    for w in extra:
                        ctr += 1
                        new_list.append(mybir.InstNoOp(
                            name=f"drainfix-{ctr}", engine=ins.engine,
                            ins=[], outs=[],
                            sync_info=mybir.SyncInfo(on_wait=[w], on_update=[])))
                    ins.sync_info.on_wait = keep
                new_list.append(ins)
            blk.instructions[:] = new_list


def _bcast(ap, ap_list):
    """Build an AP over ap's tensor with explicit [step, count] dims."""
    return bass.AP(ap.tensor, ap.offset, ap_list)


def host_prep(edge_index):
    """Pure index manipulation: assign edges to dst-owner cores, sort by dst,
    pack windows, pad, build per-core tables."""
    src = np.concatenate([edge_index[0], np.arange(N, dtype=np.int32)])
    dst = np.concatenate([edge_index[1], np.arange(N, dtype=np.int32)])
    order = np.argsort(dst, kind="stable")
    src, dst = src[order], dst[order]
    cores = []
    cap = K_TILES * 128
    for c in range(NC):
        lo, hi = c * SHARD, (c + 1) * SHARD
        m0 = np.searchsorted(dst, lo, "left")
        m1 = np.searchsorted(dst, hi, "left")
        s_c, d_c = src[m0:m1], dst[m0:m1] - lo          # dst local [0, SHARD)
        # windows: consecutive nodes, <=128 nodes, <=cap edges
        counts = np.bincount(d_c, minlength=SHARD)
        starts = np.concatenate([[0], np.cumsum(counts)])
        wins = []                                       # (node_lo, node_hi)
        n0 = 0
        while n0 < SHARD:
            n1 = n0
            while n1 < SHARD and (n1 - n0) < 128 and \
                    (starts[n1 + 1] - starts[n0]) <= cap:
                n1 += 1
            if n1 == n0:
                n1 = n0 + 1                             # single node > cap never happens
            wins.append((n0, n1))
            n0 = n1
        cores.append((s_c, d_c, starts, wins, lo))
    W = max(len(c[3]) for c in cores)
    T = W * K_TILES
    esrc = np.zeros((NC, T, 128), np.int32)
    edst = np.zeros((NC, T, 128), np.int32)
    dstrow = np.full((NC, T, 128), 999.0, np.float32)
    wnode = np.full((NC, W, 128), PAD_NODE, np.int32)
    for c, (s_c, d_c, starts, wins, lo) in enumerate(cores):
        for w, (n0, n1) in enumerate(wins):
            e0, e1 = starts[n0], starts[n1]
            ne = e1 - e0
            t0 = w * K_TILES
            flat_s = esrc[c, t0:t0 + K_TILES].reshape(-1)
            flat_d = edst[c, t0:t0 + K_TILES].reshape(-1)
            flat_r = dstrow[c, t0:t0 + K_TILES].reshape(-1)
            flat_s[:ne] = s_c[e0:e1]
            flat_d[:ne] = d_c[e0:e1] + lo               # global dst id
            flat_r[:ne] = (d_c[e0:e1] - n0).astype(np.float32)
            wnode[c, w, :n1 - n0] = np.arange(n0, n1, dtype=np.int32)
    # transpose tables to [128, T] layout for column-slice loading
    return (np.ascontiguousarray(esrc.transpose(0, 2, 1)),
            np.ascontiguousarray(edst.transpose(0, 2, 1)),
            np.ascontiguousarray(dstrow.transpose(0, 2, 1)),
            np.ascontiguousarray(wnode.transpose(0, 2, 1)), W, T)


def blockdiag(a):
    """[H, c] head vectors -> [H*c, H] block diagonal (placement only)."""
    Hh, cc = a.shape
    out = np.zeros((Hh * cc, Hh), np.float32)
    for h in range(Hh):
        out[h * cc:(h + 1) * cc, h] = a[h]
    return out


def chunk_rows(m, p=128):
    """[R, C] -> [ceil(R/p), p, C] zero-padded."""
    R, Cc = m.shape
    n = (R + p - 1) // p
    out = np.zeros((n, p, Cc), m.dtype)
    for i in range(n):
        out[i, :min(p, R - i * p)] = m[i * p:(i + 1) * p]
    return out


def build_program(W, T):
    nc = bass.Bass("TRN2")
    P = {}
    def par(name, shape, dt):
        P[name] = nc.declare_dram_parameter(name, list(shape), dt, isOutput=False)
        return P[name]

    par("xT", [12, N], DT_F32)
    par("W1", [12, F], DT_F32)
    par("W1Tc", [4, 128, 12], DT_F32)
    par("Wa1", [4, 128, 16], DT_F32)
    par("W2c", [4, 128, F], DT_F32)
    par("W2Tc", [4, 128, F], DT_F32)
    par("Wa2", [4, 128, 16], DT_F32)
    par("W3c", [4, 128, F3], DT_F32)
    par("W3T", [F3, F], DT_F32)
    par("Wa3", [F3, 16], DT_F32)
    par("b1t", [128, F], DT_F32)
    par("b2t", [128, F], DT_F32)
    par("b3t", [128, C3], DT_F32)
    par("esrc", [128, T], DT_I32)
    par("edst", [128, T], DT_I32)
    par("dstrow", [128, T], DT_F32)
    par("wnode", [128, W], DT_I32)
    OUT = nc.declare_dram_parameter("out", [SHARD, C3], DT_F32, isOutput=True)

    NT1 = (N + 127) // 128          # node tiles for replicated layer-1
    with tile.TileContext(nc) as tc:
        with (
            tc.tile_pool(name="const", bufs=1) as cp,
            tc.tile_pool(name="sbuf", bufs=3) as sb,
            tc.tile_pool(name="stage", bufs=3) as stg,
            tc.tile_pool(name="psA", bufs=2, space="PSUM") as p_A,
            tc.tile_pool(name="psB", bufs=1, space="PSUM") as p_B,
            tc.tile_pool(name="dram", bufs=1, space="DRAM") as dr,
        ):
            # ---------------- constants / weights ----------------
            ident = cp.tile([128, 128], DT_F32)
            from concourse.masks import make_identity
            make_identity(nc, ident[:])
            iota_i = cp.tile([128, 128], DT_I32)
            nc.gpsimd.iota(iota_i[:], pattern=[[1, 128]], base=0, channel_multiplier=0)
            iota_f = cp.tile([128, 128], DT_F32)
            nc.vector.tensor_copy(out=iota_f[:], in_=iota_i[:])

            t_esrc = cp.tile([128, T], DT_I32)
            nc.sync.dma_start(out=t_esrc[:], in_=P["esrc"][:])
            t_edst = cp.tile([128, T], DT_I32)
            nc.sync.dma_start(out=t_edst[:], in_=P["edst"][:])
            t_drow = cp.tile([128, T], DT_F32)
            nc.sync.dma_start(out=t_drow[:], in_=P["dstrow"][:])
            t_wn = cp.tile([128, W], DT_I32)
            nc.sync.dma_start(out=t_wn[:], in_=P["wnode"][:])
            t_b1 = cp.tile([128, F], DT_F32)
            nc.sync.dma_start(out=t_b1[:], in_=P["b1t"][:])
            t_b2 = cp.tile([128, F], DT_F32)
            nc.sync.dma_start(out=t_b2[:], in_=P["b2t"][:])
            t_b3 = cp.tile([128, C3], DT_F32)
            nc.sync.dma_start(out=t_b3[:], in_=P["b3t"][:])

            # bf16 weight copies (cast during DMA on gpsimd)
            w1 = cp.tile([12, F], DT_BF)
            nc.gpsimd.dma_start(out=w1[:], in_=P["W1"][:])
            w1T = cp.tile([128, 4, 12], DT_BF)
            w2 = cp.tile([128, 4, F], DT_BF)
            w2T = cp.tile([128, 4, F], DT_BF)
            w3 = cp.tile([128, 4, F3], DT_BF)
            wa1 = cp.tile([128, 4, 16], DT_BF)
            wa2 = cp.tile([128, 4, 16], DT_BF)
            for ch in range(4):
                nc.gpsimd.dma_start(out=w1T[:, ch, :], in_=P["W1Tc"][ch])
                nc.gpsimd.dma_start(out=w2[:, ch, :], in_=P["W2c"][ch])
                nc.gpsimd.dma_start(out=w2T[:, ch, :], in_=P["W2Tc"][ch])
                nc.gpsimd.dma_start(out=w3[:, ch, :], in_=P["W3c"][ch])
                nc.gpsimd.dma_start(out=wa1[:, ch, :], in_=P["Wa1"][ch])
                nc.gpsimd.dma_start(out=wa2[:, ch, :], in_=P["Wa2"][ch])
            w3T = cp.tile([F3, F], DT_BF)
            nc.gpsimd.dma_start(out=w3T[:], in_=P["W3T"][:])
            wa3 = cp.tile([F3, 16], DT_BF)
            nc.gpsimd.dma_start(out=wa3[:], in_=P["Wa3"][:])

            # fused attention projections  WWa_l = W_l @ Wa_l  -> [Fin_l, 16]
            wwa1 = cp.tile([12, 16], DT_BF)
            ps = p_B.tile([12, 16], DT_F32, space="PSUM", tag="pan")
            for ch in range(4):
                nc.tensor.matmul(ps[:], lhsT=w1T[:, ch, :], rhs=wa1[:, ch, :],
                                 start=(ch == 0), stop=(ch == 3))
            nc.vector.tensor_copy(out=wwa1[:], in_=ps[:])
            wwa2 = cp.tile([128, 4, 16], DT_BF)
            for fc in range(4):
                ps = p_B.tile([128, 16], DT_F32, space="PSUM", tag="pan")
                for ch in range(4):
                    nc.tensor.matmul(
                        ps[:], lhsT=w2T[:, ch, bass.ts(fc, 128)],
                        rhs=wa2[:, ch, :], start=(ch == 0), stop=(ch == 3))
                nc.vector.tensor_copy(out=wwa2[:, fc, :], in_=ps[:])
            wwa3 = cp.tile([128, 4, 16], DT_BF)
            for fc in range(4):
                ps = p_B.tile([128, 16], DT_F32, space="PSUM", tag="pan")
                nc.tensor.matmul(ps[:], lhsT=w3T[:, bass.ts(fc, 128)], rhs=wa3[:],
                                 start=True, stop=True)
                nc.vector.tensor_copy(out=wwa3[:, fc, :], in_=ps[:])

            # ---------------- DRAM internals ----------------
            OUTI = dr.tile([SHARD + 1, C3], DT_F32)
            Hf1 = dr.tile([N, F + 8], DT_BF)       # h1 | al_s1
            ALD1 = dr.tile([N, 8], DT_BF)
            exch_h = dr.tile([SHARD + 1, F + 8], DT_BF)
            exch_ad = dr.tile([SHARD + 1, 8], DT_BF)
            Hf2 = dr.tile([N, F + 8], DT_BF)
            ALD2 = dr.tile([N, 8], DT_BF)
            exch3 = dr.tile([SHARD + 1, F3 + 16], DT_F32)   # h3 | al_s3 | al_d3
            H3f = dr.tile([N, F3 + 16], DT_F32)

            # ---------------- layer-1 node phase (replicated) ----------------
            xT_sb = None
            CHT = 50                        # node tiles per xT chunk
            for t in range(NT1):
                rows = min(128, N - t * 128)
                if t % CHT == 0:
                    csz = min(CHT * 128, N - t * 128)
                    xT_sb = sb.tile([12, CHT * 128], DT_BF, tag="xT")
                    nc.gpsimd.dma_start(out=xT_sb[:, :csz],
                                        in_=P["xT"][:, t * 128:t * 128 + csz])
                off = (t % CHT) * 128
                lhs = xT_sb[:, off:off + rows]
                ph = p_A.tile([128, F], DT_F32, space="PSUM", tag="pbig2")
                nc.tensor.matmul(ph[:rows], lhsT=lhs, rhs=w1[:], start=True, stop=True)
                pa = p_B.tile([128, 16], DT_F32, space="PSUM", tag="pan")
                nc.tensor.matmul(pa[:rows], lhsT=lhs, rhs=wwa1[:], start=True, stop=True)
                hstage = stg.tile([128, F + 8], DT_BF, tag="h1s")
                nc.vector.tensor_copy(out=hstage[:rows, :F], in_=ph[:rows])
                nc.vector.tensor_copy(out=hstage[:rows, F:], in_=pa[:rows, 0:8])
                astage = stg.tile([128, 8], DT_BF, tag="a1s")
                nc.vector.tensor_copy(out=astage[:rows], in_=pa[:rows, 8:16])
                nc.sync.dma_start(out=Hf1[t * 128:t * 128 + rows, :], in_=hstage[:rows])
                nc.sync.dma_start(out=ALD1[t * 128:t * 128 + rows, :], in_=astage[:rows])

            # ---------------- edge phase (used for all 3 layers) -------------
            def edge_phase(layer, Hsrc, ALDsrc):
                lay3 = layer == 3
                FH = F3 if lay3 else F            # feature width of h
                RW = (F3 + 16) if lay3 else (F + 8)   # gathered row width
                gdt = DT_F32 if lay3 else DT_BF
                for w in range(W):
                    pden = p_A.tile([128, 8], DT_F32, space="PSUM", tag="pden")
                    pout = p_A.tile([128, FH + 8 if lay3 else FH], DT_F32,
                                      space="PSUM", tag="pout")
                    for g0 in range(0, K_TILES, G):
                        gn = min(G, K_TILES - g0)
                        tbase = w * K_TILES + g0
                        hg = stg.tile([128, G, RW], gdt, tag="hg")
                        adg = stg.tile([128, G, (F3 + 16) if lay3 else 8], gdt, tag="adg")
                        for j in range(gn):
                            nc.gpsimd.indirect_dma_start(
                                out=hg[:, j, :], out_offset=None, in_=Hsrc[:],
                                in_offset=bass.IndirectOffsetOnAxis(
                                    ap=t_esrc[:, tbase + j:tbase + j + 1], axis=0))
                            nc.gpsimd.indirect_dma_start(
                                out=adg[:, j, :], out_offset=None,
                                in_=Hsrc[:] if lay3 else ALDsrc[:],
                                in_offset=bass.IndirectOffsetOnAxis(
                                    ap=t_edst[:, tbase + j:tbase + j + 1], axis=0))
                        # e = al_s[src] + al_d[dst]; alpha-num = exp(lrelu(e))
                        als_off = F3 + 8 if lay3 else F
                        e_t = sb.tile([128, G, 8], DT_F32, tag="e")
                        nc.vector.tensor_tensor(
                            out=e_t[:, :gn, :], in0=hg[:, :gn, als_off:als_off + 8],
                            in1=adg[:, :gn, F3 + 8:F3 + 16] if lay3 else adg[:, :gn, :],
                            op=ALU.add)
                        lr = sb.tile([128, G, 8], DT_F32, tag="lr")
                        nc.scalar.activation(lr[:, :gn, :], e_t[:, :gn, :], AF.Lrelu,
                                             alpha=NEG_SLOPE)
                        ex = sb.tile([128, G, 8], DT_F32, tag="ex")
                        nc.scalar.activation(ex[:, :gn, :], lr[:, :gn, :], AF.Exp)
                        exb = sb.tile([128, G, 8], DT_BF, tag="exb")
                        nc.vector.tensor_copy(out=exb[:, :gn, :], in_=ex[:, :gn, :])
                        # msg = h_gathered * ex  (broadcast over channel dim)
                        CW = C3 if lay3 else C
                        msg = sb.tile([128, G, FH + 8 if lay3 else FH], DT_BF, tag="msg")
                        ex_b4 = _bcast(ex[:], [ex[:].ap[0], [8, gn], [1, 8], [0, CW]])
                        hg4 = _bcast(hg[:], [hg[:].ap[0], [RW, gn], [CW, 8], [1, CW]])
                        msg4 = _bcast(msg[:], [msg[:].ap[0],
                                               [FH + 8 if lay3 else FH, gn], [CW, 8], [1, CW]])
                        nc.vector.tensor_tensor(out=msg4, in0=hg4, in1=ex_b4, op=ALU.mult)
                        if lay3:
                            nc.vector.tensor_copy(out=msg[:, :gn, F3:F3 + 8],
                                                  in_=exb[:, :gn, :])
                        # selection matrices
                        sel = sb.tile([128, G, 128], DT_BF, tag="sel")
                        drow_b = _bcast(t_drow[:, tbase:tbase + gn],
                                        [t_drow[:].ap[0], [1, gn], [0, 128]])
                        iota_b = _bcast(iota_f[:], [iota_f[:].ap[0], [0, gn], [1, 128]])
                        nc.vector.tensor_tensor(out=sel[:, :gn, :], in0=drow_b,
                                                in1=iota_b, op=ALU.is_equal)
                        first = g0 == 0
                        last_g = g0 + gn == K_TILES
                        for j in range(gn):
                            st = first and j == 0
                            sp = last_g and j == gn - 1
                            nc.tensor.matmul(pout[:], lhsT=sel[:, j, :],
                                             rhs=msg[:, j, :], start=st, stop=sp)
                            if not lay3:
                                nc.tensor.matmul(pden[:], lhsT=sel[:, j, :],
                                                 rhs=exb[:, j, :], start=st, stop=sp)
                    # ---- window close: normalize, bias, next-layer ----
                    den = sb.tile([128, 8], DT_F32, tag="den")
                    if lay3:
                        nc.vector.tensor_scalar_add(den[:], pout[:, F3:F3 + 8], 1e-16)
                    else:
                        nc.vector.tensor_scalar_add(den[:], pden[:], 1e-16)
                    rec = sb.tile([128, 8], DT_F32, tag="rec")
                    nc.vector.reciprocal(rec[:], den[:])
                    onrm = sb.tile([128, FH], DT_F32, tag="onrm")
                    CW = C3 if lay3 else C
                    rec_b = _bcast(rec[:], [rec[:].ap[0], [1, 8], [0, CW]])
                    po4 = _bcast(pout[:], [pout[:].ap[0], [CW, 8], [1, CW]])
                    on4 = _bcast(onrm[:], [onrm[:].ap[0], [CW, 8], [1, CW]])
                    nc.vector.tensor_tensor(out=on4, in0=po4, in1=rec_b, op=ALU.mult)
                    wn_ap = t_wn[:, w:w + 1]
                    if lay3:
                        hm = sb.tile([128, C3], DT_F32, tag="hm")
                        on_T = _bcast(onrm[:], [onrm[:].ap[0], [1, C3], [C3, 8]])
                        nc.vector.reduce_sum(hm[:], on_T, axis=mybir.AxisListType.X)
                        nc.vector.tensor_scalar_mul(hm[:], hm[:], 0.125)
                        nc.vector.tensor_add(out=hm[:], in0=hm[:], in1=t_b3[:])
                        mx = sb.tile([128, 1], DT_F32, tag="mx")
                        nc.vector.reduce_max(mx[:], hm[:], axis=mybir.AxisListType.X)
                        xc = sb.tile([128, C3], DT_F32, tag="xc")
                        nc.vector.tensor_tensor(out=xc[:], in0=hm[:],
                                                in1=mx[:].to_broadcast([128, C3]),
                                                op=ALU.subtract)
                        e5 = sb.tile([128, C3], DT_F32, tag="e5")
                        nc.scalar.activation(e5[:], xc[:], AF.Exp)
                        s5 = sb.tile([128, 1], DT_F32, tag="s5")
                        nc.vector.reduce_sum(s5[:], e5[:], axis=mybir.AxisListType.X)
                        lg = sb.tile([128, 1], DT_F32, tag="lg")
                        nc.scalar.activation(lg[:], s5[:], AF.Ln)
                        res = sb.tile([128, C3], DT_F32, tag="res")
                        nc.vector.tensor_tensor(out=res[:], in0=xc[:],
                                                in1=lg[:].to_broadcast([128, C3]),
                                                op=ALU.subtract)
                        nc.gpsimd.indirect_dma_start(
                            out=OUTI[:], out_offset=bass.IndirectOffsetOnAxis(
                                ap=wn_ap, axis=0),
                            in_=res[:], in_offset=None)
                        continue
                    # bias + relu -> x_next
                    nc.vector.tensor_add(out=onrm[:], in0=onrm[:],
                                         in1=t_b1[:] if layer == 1 else t_b2[:])
                    xn = sb.tile([128, F], DT_F32, tag="xn")
                    nc.scalar.activation(xn[:], onrm[:], AF.Relu)
                    # transpose x_next -> lhsT chunks
                    xnT = sb.tile([128, 4, 128], DT_BF, tag="xnT")
                    for ch in range(4):
                        pt = p_B.tile([128, 128], DT_F32, space="PSUM", tag="ptr")
                        nc.tensor.transpose(pt[:], xn[:, bass.ts(ch, 128)], ident[:])
                        nc.vector.tensor_copy(out=xnT[:, ch, :], in_=pt[:])
                    # next-layer h / al
                    wN = w2 if layer == 1 else w3
                    wwaN = wwa2 if layer == 1 else wwa3
                    FN = F if layer == 1 else F3
                    ph = p_A.tile([128, FN], DT_F32, space="PSUM", tag="pbig2")
                    pa = p_B.tile([128, 16], DT_F32, space="PSUM", tag="pan")
                    for ch in range(4):
                        nc.tensor.matmul(ph[:], lhsT=xnT[:, ch, :], rhs=wN[:, ch, :],
                                         start=(ch == 0), stop=(ch == 3))
                        nc.tensor.matmul(pa[:], lhsT=xnT[:, ch, :], rhs=wwaN[:, ch, :],
                                         start=(ch == 0), stop=(ch == 3))
                    if layer == 1:
                        hstage = stg.tile([128, F + 8], DT_BF, tag="h2s")
                        nc.vector.tensor_copy(out=hstage[:, :F], in_=ph[:])
                        nc.vector.tensor_copy(out=hstage[:, F:], in_=pa[:, 0:8])
                        astage = stg.tile([128, 8], DT_BF, tag="a2s")
                        nc.vector.tensor_copy(out=astage[:], in_=pa[:, 8:16])
                        nc.gpsimd.indirect_dma_start(
                            out=exch_h[:], out_offset=bass.IndirectOffsetOnAxis(
                                ap=wn_ap, axis=0),
                            in_=hstage[:], in_offset=None)
                        nc.gpsimd.indirect_dma_start(
                            out=exch_ad[:], out_offset=bass.IndirectOffsetOnAxis(
                                ap=wn_ap, axis=0),
                            in_=astage[:], in_offset=None)
                    else:
                        h3stage = stg.tile([128, F3 + 16], DT_F32, tag="h3s")
                        nc.vector.tensor_copy(out=h3stage[:, :F3], in_=ph[:])
                        nc.vector.tensor_copy(out=h3stage[:, F3:], in_=pa[:])
                        nc.gpsimd.indirect_dma_start(
                            out=exch3[:], out_offset=bass.IndirectOffsetOnAxis(
                                ap=wn_ap, axis=0),
                            in_=h3stage[:], in_offset=None)

            edge_phase(1, Hf1, ALD1)
            rg = [list(range(NC))]
            nc.gpsimd.collective_compute("AllGather", ALU.bypass, replica_groups=rg,
                                         ins=[exch_h[0:SHARD, :].opt()], outs=[Hf2[:].opt()])
            nc.gpsimd.collective_compute("AllGather", ALU.bypass, replica_groups=rg,
                                         ins=[exch_ad[0:SHARD, :].opt()], outs=[ALD2[:].opt()])
            edge_phase(2, Hf2, ALD2)
            nc.gpsimd.collective_compute("AllGather", ALU.bypass, replica_groups=rg,
                                         ins=[exch3[0:SHARD, :].opt()], outs=[H3f[:].opt()])
            edge_phase(3, H3f, None)
            nc.sync.dma_start(out=OUT[:], in_=OUTI[0:SHARD, :])

    _split_drain_waits(nc)
    return nc


_CACHE = {}


def kernel(**inputs):
    x = np.asarray(inputs["x"], np.float32)
    edge_index = np.asarray(inputs["edge_index"], np.int32)
    esrc, edst, dstrow, wnode, W, T = host_prep(edge_index)
    key = (W, T)
    if key not in _CACHE:
        _CACHE[key] = build_program(W, T)
    nc = _CACHE[key]

    com = {
        "xT": np.ascontiguousarray(x.T),
        "W1": np.asarray(inputs["W1"], np.float32),
        "W1Tc": chunk_rows(np.ascontiguousarray(np.asarray(inputs["W1"]).T)),
        "Wa1": chunk_rows(np.concatenate(
            [blockdiag(np.asarray(inputs["as1"])), blockdiag(np.asarray(inputs["ad1"]))], 1)),
        "W2c": chunk_rows(np.asarray(inputs["W2"], np.float32)),
        "W2Tc": chunk_rows(np.ascontiguousarray(np.asarray(inputs["W2"]).T)),
        "Wa2": chunk_rows(np.concatenate(
            [blockdiag(np.asarray(inputs["as2"])), blockdiag(np.asarray(inputs["ad2"]))], 1)),
        "W3c": chunk_rows(np.asarray(inputs["W3"], np.float32)),
        "W3T": np.ascontiguousarray(np.asarray(inputs["W3"]).T),
        "Wa3": np.concatenate(
            [blockdiag(np.asarray(inputs["as3"])), blockdiag(np.asarray(inputs["ad3"]))], 1),
        "b1t": np.tile(np.asarray(inputs["b1"], np.float32)[None, :], (128, 1)),
        "b2t": np.tile(np.asarray(inputs["b2"], np.float32)[None, :], (128, 1)),
        "b3t": np.tile(np.asarray(inputs["b3"], np.float32)[None, :], (128, 1)),
    }
    in_maps = []
    for c in range(NC):
        m = dict(com)
        m["esrc"] = esrc[c]
        m["edst"] = edst[c]
        m["dstrow"] = dstrow[c]
        m["wnode"] = wnode[c]
        in_maps.append(m)
    res = run_bass_kernel_spmd(nc, in_maps, list(range(NC)))
    return np.concatenate([res.results[c]["out"] for c in range(NC)], axis=0)


# revision 9
# speedup vs baseline: 379.6305x; 379.6305x over previous
"""3-layer GAT (GATConv x3 + log_softmax) on 8 trn2 NeuronCores.

Strategy: 1-D node partition (6250 nodes/core). Edges live on the core that
owns their destination node, sorted by dst and packed into fixed-size
"windows" (<=128 dst nodes, K edge-tiles of 128 edges). Segment softmax +
scatter-add are done with per-tile one-hot selection matmuls accumulating in
PSUM. Node features for layer l+1 are produced per-window (fused projection)
and exchanged with an AllGather collective; layer-1 features are computed
replicated (Fin=12, cheap) so only two exchanges are needed.
"""
import numpy as np
import ml_dtypes

import concourse.bass as bass
import concourse.mybir as mybir
import concourse.tile as tile
from concourse.bass_utils import run_bass_kernel_spmd

BF = ml_dtypes.bfloat16
N = 50000
NC = 8
SHARD = N // NC            # 6250
H, C = 8, 64
F = H * C                  # 512
C3 = 5
F3 = H * C3                # 40
K_TILES = 8                # edge tiles per window (128 edges each)
G = 4                      # tiles per inner group
PAD_NODE = SHARD  # trash row
NEG_SLOPE = 0.2
DT_BF = mybir.dt.bfloat16
DT_F32 = mybir.dt.float32
DT_I32 = mybir.dt.int32
AF = mybir.ActivationFunctionType
ALU = mybir.AluOpType


def _split_drain_waits(nc, max_waits=1):
    # walrus on this toolchain rejects instructions carrying more than a few
    # sync waits; keep <=max_waits per instruction, move extras onto NoOps
    # inserted just before (same engine -> executes first, semantics kept).
    ctr = 0
    for f in nc.m.functions:
        for blk in f.blocks:
            new_list = []
            for ins in blk.instructions:
                if ins.sync_info and \
                        len(ins.sync_info.on_wait) > max_waits:
                    waits = list(ins.sync_info.on_wait)
                    keep, extra = waits[:max_waits], waits[max_waits:]
                    for w in extra:
                        ctr += 1
                        new_list.append(mybir.InstNoOp(
                            name=f"drainfix-{ctr}", engine=ins.engine,
                            ins=[], outs=[],
                            sync_info=mybir.SyncInfo(on_wait=[w], on_update=[])))
                    ins.sync_info.on_wait = keep
                new_list.append(ins)
            blk.instructions[:] = new_list


def _bcast(ap, ap_list):
    """Build an AP over ap's tensor with explicit [step, count] dims."""
    return bass.AP(ap.tensor, ap.offset, ap_list)


def host_prep(edge_index):
    """Pure index manipulation: assign edges to dst-owner cores, sort by dst,
    pack windows, pad, build per-core tables."""
    src = np.concatenate([edge_index[0], np.arange(N, dtype=np.int32)])
    dst = np.concatenate([edge_index[1], np.arange(N, dtype=np.int32)])
    order = np.argsort(dst, kind="stable")
    src, dst = src[order], dst[order]
    cores = []
    cap = K_TILES * 128
    for c in range(NC):
        lo, hi = c * SHARD, (c + 1) * SHARD
        m0 = np.searchsorted(dst, lo, "left")
        m1 = np.searchsorted(dst, hi, "left")
        s_c, d_c = src[m0:m1], dst[m0:m1] - lo          # dst local [0, SHARD)
        # windows: consecutive nodes, <=128 nodes, <=cap edges
        counts = np.bincount(d_c, minlength=SHARD)
        starts = np.concatenate([[0], np.cumsum(counts)])
        wins = []                                       # (node_lo, node_hi)
        n0 = 0
        while n0 < SHARD:
            n1 = n0
            while n1 < SHARD and (n1 - n0) < 128 and \
                    (starts[n1 + 1] - starts[n0]) <= cap:
                n1 += 1
            if n1 == n0:
                n1 = n0 + 1                             # single node > cap never happens
            wins.append((n0, n1))
            n0 = n1
        cores.append((s_c, d_c, starts, wins, lo))
    W = max(len(c[3]) for c in cores)
    T = W * K_TILES
    esrc = np.zeros((NC, T, 128), np.int32)
    edst = np.zeros((NC, T, 128), np.int32)
    dstrow = np.full((NC, T, 128), 999.0, np.float32)
    wnode = np.full((NC, W, 128), PAD_NODE, np.int32)
    wnodeg = np.zeros((NC, W, 128), np.int32)
    for c, (s_c, d_c, starts, wins, lo) in enumerate(cores):
        for w, (n0, n1) in enumerate(wins):
            e0, e1 = starts[n0], starts[n1]
            ne = e1 - e0
            t0 = w * K_TILES
            flat_s = esrc[c, t0:t0 + K_TILES].reshape(-1)
            flat_d = edst[c, t0:t0 + K_TILES].reshape(-1)
            flat_r = dstrow[c, t0:t0 + K_TILES].reshape(-1)
            flat_s[:ne] = s_c[e0:e1]
            flat_d[:ne] = d_c[e0:e1] + lo               # global dst id
            flat_r[:ne] = (d_c[e0:e1] - n0).astype(np.float32)
            wnode[c, w, :n1 - n0] = np.arange(n0, n1, dtype=np.int32)
            wnodeg[c, w, :n1 - n0] = np.arange(n0, n1, dtype=np.int32) + lo
    # transpose tables to [128, T] layout for column-slice loading
    return (np.ascontiguousarray(esrc.transpose(0, 2, 1)),
            np.ascontiguousarray(edst.transpose(0, 2, 1)),
            np.ascontiguousarray(dstrow.transpose(0, 2, 1)),
            np.ascontiguousarray(wnode.transpose(0, 2, 1)),
            np.ascontiguousarray(wnodeg.transpose(0, 2, 1)), W, T)


def blockdiag(a):
    """[H, c] head vectors -> [H*c, H] block diagonal (placement only)."""
    Hh, cc = a.shape
    out = np.zeros((Hh * cc, Hh), np.float32)
    for h in range(Hh):
        out[h * cc:(h + 1) * cc, h] = a[h]
    return out


def chunk_rows(m, p=128):
    """[R, C] -> [ceil(R/p), p, C] zero-padded."""
    R, Cc = m.shape
    n = (R + p - 1) // p
    out = np.zeros((n, p, Cc), m.dtype)
    for i in range(n):
        out[i, :min(p, R - i * p)] = m[i * p:(i + 1) * p]
    return out


def build_program(W, T):
    nc = bass.Bass("TRN2")
    P = {}
    def par(name, shape, dt):
        P[name] = nc.declare_dram_parameter(name, list(shape), dt, isOutput=False)
        return P[name]

    par("xT", [12, N], DT_F32)
    par("W1", [12, F], DT_F32)
    par("W1Tc", [4, 128, 12], DT_F32)
    par("Wa1", [4, 128, 16], DT_F32)
    par("W2c", [4, 128, F], DT_F32)
    par("W2Tc", [4, 128, F], DT_F32)
    par("Wa2", [4, 128, 16], DT_F32)
    par("W3c", [4, 128, F3], DT_F32)
    par("W3T", [F3, F], DT_F32)
    par("Wa3", [F3, 16], DT_F32)
    par("b1t", [128, F], DT_F32)
    par("b2t", [128, F], DT_F32)
    par("b3t", [128, C3], DT_F32)
    par("esrc", [128, T], DT_I32)
    par("edst", [128, T], DT_I32)
    par("dstrow", [128, T], DT_F32)
    par("wnode", [128, W], DT_I32)
    par("wnodeg", [128, W], DT_I32)
    OUT = nc.declare_dram_parameter("out", [SHARD, C3], DT_F32, isOutput=True)
    par("tick", [128, 1], DT_F32)
    TOCK = nc.declare_dram_parameter("tock", [128, 1], DT_F32, isOutput=True)

    NT1 = (N + 127) // 128          # node tiles for replicated layer-1
    with tile.TileContext(nc) as tc:
        with (
            tc.tile_pool(name="const", bufs=1) as cp,
            tc.tile_pool(name="sbuf", bufs=3) as sb,
            tc.tile_pool(name="stage", bufs=3) as stg,
            tc.tile_pool(name="psA", bufs=2, space="PSUM") as p_A,
            tc.tile_pool(name="psB", bufs=1, space="PSUM") as p_B,
            tc.tile_pool(name="dram", bufs=1, space="DRAM") as dr,
        ):
            # ---------------- constants / weights ----------------
            ident = cp.tile([128, 128], DT_F32)
            from concourse.masks import make_identity
            make_identity(nc, ident[:])
            ident_bf = cp.tile([128, 128], DT_BF)
            nc.vector.tensor_copy(out=ident_bf[:], in_=ident[:])
            iota_i = cp.tile([128, 128], DT_I32)
            nc.gpsimd.iota(iota_i[:], pattern=[[1, 128]], base=0, channel_multiplier=0)
            iota_f = cp.tile([128, 128], DT_F32)
            nc.vector.tensor_copy(out=iota_f[:], in_=iota_i[:])

            t_esrc = cp.tile([128, T], DT_I32)
            nc.sync.dma_start(out=t_esrc[:], in_=P["esrc"][:])
            t_edst = cp.tile([128, T], DT_I32)
            nc.sync.dma_start(out=t_edst[:], in_=P["edst"][:])
            t_drow = cp.tile([128, T], DT_F32)
            nc.sync.dma_start(out=t_drow[:], in_=P["dstrow"][:])
            t_wn = cp.tile([128, W], DT_I32)
            nc.sync.dma_start(out=t_wn[:], in_=P["wnode"][:])
            t_wng = cp.tile([128, W], DT_I32)
            nc.sync.dma_start(out=t_wng[:], in_=P["wnodeg"][:])
            t_b1 = cp.tile([128, F], DT_F32)
            nc.sync.dma_start(out=t_b1[:], in_=P["b1t"][:])
            t_b2 = cp.tile([128, F], DT_F32)
            nc.sync.dma_start(out=t_b2[:], in_=P["b2t"][:])
            t_b3 = cp.tile([128, C3], DT_F32)
            nc.sync.dma_start(out=t_b3[:], in_=P["b3t"][:])

            # bf16 weight copies (cast during DMA on gpsimd)
            w1 = cp.tile([12, F], DT_BF)
            nc.gpsimd.dma_start(out=w1[:], in_=P["W1"][:])
            w1T = cp.tile([128, 4, 12], DT_BF)
            w2 = cp.tile([128, 4, F], DT_BF)
            w2T = cp.tile([128, 4, F], DT_BF)
            w3 = cp.tile([128, 4, F3], DT_BF)
            wa1 = cp.tile([128, 4, 16], DT_BF)
            wa2 = cp.tile([128, 4, 16], DT_BF)
            for ch in range(4):
                nc.gpsimd.dma_start(out=w1T[:, ch, :], in_=P["W1Tc"][ch])
                nc.gpsimd.dma_start(out=w2[:, ch, :], in_=P["W2c"][ch])
                nc.gpsimd.dma_start(out=w2T[:, ch, :], in_=P["W2Tc"][ch])
                nc.gpsimd.dma_start(out=w3[:, ch, :], in_=P["W3c"][ch])
                nc.gpsimd.dma_start(out=wa1[:, ch, :], in_=P["Wa1"][ch])
                nc.gpsimd.dma_start(out=wa2[:, ch, :], in_=P["Wa2"][ch])
            w3T = cp.tile([F3, F], DT_BF)
            nc.gpsimd.dma_start(out=w3T[:], in_=P["W3T"][:])
            wa3 = cp.tile([F3, 16], DT_BF)
            nc.gpsimd.dma_start(out=wa3[:], in_=P["Wa3"][:])

            # fused attention projections  WWa_l = W_l @ Wa_l  -> [Fin_l, 16]
            wwa1 = cp.tile([12, 16], DT_BF)
            ps = p_B.tile([12, 16], DT_F32, space="PSUM", tag="pan")
            for ch in range(4):
                nc.tensor.matmul(ps[:], lhsT=w1T[:, ch, :], rhs=wa1[:, ch, :],
                                 start=(ch == 0), stop=(ch == 3))
            nc.vector.tensor_copy(out=wwa1[:], in_=ps[:])
            wwa2 = cp.tile([128, 4, 16], DT_BF)
            for fc in range(4):
                ps = p_B.tile([128, 16], DT_F32, space="PSUM", tag="pan")
                for ch in range(4):
                    nc.tensor.matmul(
                        ps[:], lhsT=w2T[:, ch, bass.ts(fc, 128)],
                        rhs=wa2[:, ch, :], start=(ch == 0), stop=(ch == 3))
                nc.vector.tensor_copy(out=wwa2[:, fc, :], in_=ps[:])
            wwa3 = cp.tile([128, 4, 16], DT_BF)
            for fc in range(4):
                ps = p_B.tile([128, 16], DT_F32, space="PSUM", tag="pan")
                nc.tensor.matmul(ps[:], lhsT=w3T[:, bass.ts(fc, 128)], rhs=wa3[:],
                                 start=True, stop=True)
                nc.vector.tensor_copy(out=wwa3[:, fc, :], in_=ps[:])

            # ---------------- DRAM internals ----------------
            OUTI = dr.tile([SHARD + 1, C3], DT_F32)
            Hf1 = dr.tile([N, F + 8], DT_BF)       # h1 | al_s1
            ALD1 = dr.tile([N, 8], DT_BF)
            exch_h = dr.tile([SHARD + 1, F + 8], DT_BF)
            exch_ad = dr.tile([SHARD + 1, 8], DT_BF)
            Hf2 = dr.tile([N, F + 8], DT_BF)
            exch3 = dr.tile([SHARD + 1, F3 + 16], DT_F32)   # h3 | al_s3 | al_d3
            H3f = dr.tile([N, F3 + 16], DT_F32)

            # ---------------- layer-1 node phase (replicated) ----------------
            xT_sb = None
            CHT = 50                        # node tiles per xT chunk
            for t in range(NT1):
                rows = min(128, N - t * 128)
                if t % CHT == 0:
                    csz = min(CHT * 128, N - t * 128)
                    xT_sb = sb.tile([12, CHT * 128], DT_BF, tag="xT")
                    nc.gpsimd.dma_start(out=xT_sb[:, :csz],
                                        in_=P["xT"][:, t * 128:t * 128 + csz])
                off = (t % CHT) * 128
                lhs = xT_sb[:, off:off + rows]
                ph = p_A.tile([128, F], DT_F32, space="PSUM", tag="pbig2")
                nc.tensor.matmul(ph[:rows], lhsT=lhs, rhs=w1[:], start=True, stop=True)
                pa = p_B.tile([128, 16], DT_F32, space="PSUM", tag="pan")
                nc.tensor.matmul(pa[:rows], lhsT=lhs, rhs=wwa1[:], start=True, stop=True)
                hstage = stg.tile([128, F + 8], DT_BF, tag="h1s")
                nc.vector.tensor_copy(out=hstage[:rows, :F], in_=ph[:rows])
                nc.vector.tensor_copy(out=hstage[:rows, F:], in_=pa[:rows, 0:8])
                astage = stg.tile([128, 8], DT_BF, tag="a1s")
                nc.vector.tensor_copy(out=astage[:rows], in_=pa[:rows, 8:16])
                nc.sync.dma_start(out=Hf1[t * 128:t * 128 + rows, :], in_=hstage[:rows])
                nc.sync.dma_start(out=ALD1[t * 128:t * 128 + rows, :], in_=astage[:rows])

            # ---------------- edge phase (used for all 3 layers) -------------
            def edge_phase(layer, Hsrc, ALDsrc, Hsrc_local=None):
                lay3 = layer == 3
                FH = F3 if lay3 else F            # feature width of h
                RW = (F3 + 16) if lay3 else (F + 8)   # gathered row width
                gdt = DT_F32 if lay3 else DT_BF
                for w in range(W):
                    pden = p_B.tile([128, 8], DT_F32, space="PSUM", tag="pden")
                    pout = p_A.tile([128, FH + 8 if lay3 else FH], DT_F32,
                                      space="PSUM", tag="pout")
                    # window al_d: one gather for the <=128 dst nodes
                    adw = stg.tile([128, (F3 + 16) if lay3 else 8], gdt, tag="adw")
                    nc.gpsimd.indirect_dma_start(
                        out=adw[:], out_offset=None,
                        in_=ALDsrc[:] if not lay3 else Hsrc_local[:],
                        in_offset=bass.IndirectOffsetOnAxis(
                            ap=(t_wng if layer == 1 else t_wn)[:, w:w + 1], axis=0))
                    adw_b = sb.tile([128, 8], DT_BF, tag="adwb")
                    nc.vector.tensor_copy(
                        out=adw_b[:], in_=adw[:, F3 + 8:F3 + 16] if lay3 else adw[:])
                    for g0 in range(0, K_TILES, G):
                        gn = min(G, K_TILES - g0)
                        tbase = w * K_TILES + g0
                        hg = stg.tile([128, G, RW], gdt, tag="hg")
                        for j in range(gn):
                            nc.gpsimd.indirect_dma_start(
                                out=hg[:, j, :], out_offset=None, in_=Hsrc[:],
                                in_offset=bass.IndirectOffsetOnAxis(
                                    ap=t_esrc[:, tbase + j:tbase + j + 1], axis=0))
                        # selection matrices + their transposes (for al_d expand)
                        sel = sb.tile([128, G, 128], DT_BF, tag="sel")
                        drow_b = _bcast(t_drow[:, tbase:tbase + gn],
                                        [t_drow[:].ap[0], [1, gn], [0, 128]])
                        iota_b = _bcast(iota_f[:], [iota_f[:].ap[0], [0, gn], [1, 128]])
                        nc.vector.tensor_tensor(out=sel[:, :gn, :], in0=drow_b,
                                                in1=iota_b, op=ALU.is_equal)
                        selT = sb.tile([128, G, 128], DT_BF, tag="selT")
                        pad_ps = p_B.tile([128, G * 8], DT_F32, space="PSUM", tag="pad")
                        for j in range(gn):
                            ptx = p_B.tile([128, 128], DT_BF, space="PSUM", tag="ptr")
                            nc.tensor.transpose(ptx[:], sel[:, j, :], ident_bf[:])
                            nc.vector.tensor_copy(out=selT[:, j, :], in_=ptx[:])
                            nc.tensor.matmul(pad_ps[:, j * 8:(j + 1) * 8],
                                             lhsT=selT[:, j, :], rhs=adw_b[:],
                                             start=True, stop=True)
                        # e = al_s[src] + al_d[dst]; alpha-num = exp(lrelu(e))
                        als_off = F3 + 8 if lay3 else F
                        e_t = sb.tile([128, G, 8], DT_F32, tag="e")
                        pad_v = _bcast(pad_ps[:], [pad_ps[:].ap[0], [8, gn], [1, 8]])
                        nc.vector.tensor_tensor(
                            out=e_t[:, :gn, :], in0=hg[:, :gn, als_off:als_off + 8],
                            in1=pad_v, op=ALU.add)
                        lr = sb.tile([128, G, 8], DT_F32, tag="lr")
                        nc.scalar.activation(lr[:, :gn, :], e_t[:, :gn, :], AF.Lrelu,
                                             alpha=NEG_SLOPE)
                        ex = sb.tile([128, G, 8], DT_F32, tag="ex")
                        nc.scalar.activation(ex[:, :gn, :], lr[:, :gn, :], AF.Exp)
                        exb = sb.tile([128, G, 8], DT_BF, tag="exb")
                        nc.vector.tensor_copy(out=exb[:, :gn, :], in_=ex[:, :gn, :])
                        # msg = h_gathered * ex  (broadcast over channel dim)
                        CW = C3 if lay3 else C
                        msg = sb.tile([128, G, FH + 8 if lay3 else FH], DT_BF, tag="msg")
                        ex_b4 = _bcast(ex[:], [ex[:].ap[0], [8, gn], [1, 8], [0, CW]])
                        hg4 = _bcast(hg[:], [hg[:].ap[0], [RW, gn], [CW, 8], [1, CW]])
                        msg4 = _bcast(msg[:], [msg[:].ap[0],
                                               [FH + 8 if lay3 else FH, gn], [CW, 8], [1, CW]])
                        nc.vector.tensor_tensor(out=msg4, in0=hg4, in1=ex_b4, op=ALU.mult)
                        if lay3:
                            nc.vector.tensor_copy(out=msg[:, :gn, F3:F3 + 8],
                                                  in_=exb[:, :gn, :])
                        first = g0 == 0
                        last_g = g0 + gn == K_TILES
                        for j in range(gn):
                            st = first and j == 0
                            sp = last_g and j == gn - 1
                            nc.tensor.matmul(pout[:], lhsT=sel[:, j, :],
                                             rhs=msg[:, j, :], start=st, stop=sp)
                            if not lay3:
                                nc.tensor.matmul(pden[:], lhsT=sel[:, j, :],
                                                 rhs=exb[:, j, :], start=st, stop=sp)
                    # ---- window close: normalize, bias, next-layer ----
                    den = sb.tile([128, 8], DT_F32, tag="den")
                    if lay3:
                        nc.vector.tensor_scalar_add(den[:], pout[:, F3:F3 + 8], 1e-16)
                    else:
                        nc.vector.tensor_scalar_add(den[:], pden[:], 1e-16)
                    rec = sb.tile([128, 8], DT_F32, tag="rec")
                    nc.vector.reciprocal(rec[:], den[:])
                    onrm = sb.tile([128, FH], DT_F32, tag="onrm")
                    CW = C3 if lay3 else C
                    rec_b = _bcast(rec[:], [rec[:].ap[0], [1, 8], [0, CW]])
                    po4 = _bcast(pout[:], [pout[:].ap[0], [CW, 8], [1, CW]])
                    on4 = _bcast(onrm[:], [onrm[:].ap[0], [CW, 8], [1, CW]])
                    nc.vector.tensor_tensor(out=on4, in0=po4, in1=rec_b, op=ALU.mult)
                    wn_ap = t_wn[:, w:w + 1]
                    if lay3:
                        hm = sb.tile([128, C3], DT_F32, tag="hm")
                        on_T = _bcast(onrm[:], [onrm[:].ap[0], [1, C3], [C3, 8]])
                        nc.vector.reduce_sum(hm[:], on_T, axis=mybir.AxisListType.X)
                        nc.vector.tensor_scalar_mul(hm[:], hm[:], 0.125)
                        nc.vector.tensor_add(out=hm[:], in0=hm[:], in1=t_b3[:])
                        mx = sb.tile([128, 1], DT_F32, tag="mx")
                        nc.vector.reduce_max(mx[:], hm[:], axis=mybir.AxisListType.X)
                        xc = sb.tile([128, C3], DT_F32, tag="xc")
                        nc.vector.tensor_tensor(out=xc[:], in0=hm[:],
                                                in1=mx[:].to_broadcast([128, C3]),
                                                op=ALU.subtract)
                        e5 = sb.tile([128, C3], DT_F32, tag="e5")
                        nc.scalar.activation(e5[:], xc[:], AF.Exp)
                        s5 = sb.tile([128, 1], DT_F32, tag="s5")
                        nc.vector.reduce_sum(s5[:], e5[:], axis=mybir.AxisListType.X)
                        lg = sb.tile([128, 1], DT_F32, tag="lg")
                        nc.scalar.activation(lg[:], s5[:], AF.Ln)
                        res = sb.tile([128, C3], DT_F32, tag="res")
                        nc.vector.tensor_tensor(out=res[:], in0=xc[:],
                                                in1=lg[:].to_broadcast([128, C3]),
                                                op=ALU.subtract)
                        nc.gpsimd.indirect_dma_start(
                            out=OUTI[:], out_offset=bass.IndirectOffsetOnAxis(
                                ap=wn_ap, axis=0),
                            in_=res[:], in_offset=None)
                        continue
                    # bias + relu -> x_next
                    nc.vector.tensor_add(out=onrm[:], in0=onrm[:],
                                         in1=t_b1[:] if layer == 1 else t_b2[:])
                    xn = sb.tile([128, F], DT_F32, tag="xn")
                    nc.scalar.activation(xn[:], onrm[:], AF.Relu)
                    # transpose x_next -> lhsT chunks
                    xnT = sb.tile([128, 4, 128], DT_BF, tag="xnT")
                    for ch in range(4):
                        pt = p_B.tile([128, 128], DT_F32, space="PSUM", tag="ptr")
                        nc.tensor.transpose(pt[:], xn[:, bass.ts(ch, 128)], ident[:])
                        nc.vector.tensor_copy(out=xnT[:, ch, :], in_=pt[:])
                    # next-layer h / al
                    wN = w2 if layer == 1 else w3
                    wwaN = wwa2 if layer == 1 else wwa3
                    FN = F if layer == 1 else F3
                    ph = p_A.tile([128, FN], DT_F32, space="PSUM", tag="pbig2")
                    pa = p_B.tile([128, 16], DT_F32, space="PSUM", tag="pan")
                    for ch in range(4):
                        nc.tensor.matmul(ph[:], lhsT=xnT[:, ch, :], rhs=wN[:, ch, :],
                                         start=(ch == 0), stop=(ch == 3))
                        nc.tensor.matmul(pa[:], lhsT=xnT[:, ch, :], rhs=wwaN[:, ch, :],
                                         start=(ch == 0), stop=(ch == 3))
                    if layer == 1:
                        hstage = stg.tile([128, F + 8], DT_BF, tag="h2s")
                        nc.vector.tensor_copy(out=hstage[:, :F], in_=ph[:])
                        nc.vector.tensor_copy(out=hstage[:, F:], in_=pa[:, 0:8])
                        astage = stg.tile([128, 8], DT_BF, tag="a2s")
                        nc.vector.tensor_copy(out=astage[:], in_=pa[:, 8:16])
                        nc.gpsimd.indirect_dma_start(
                            out=exch_h[:], out_offset=bass.IndirectOffsetOnAxis(
                                ap=wn_ap, axis=0),
                            in_=hstage[:], in_offset=None)
                        nc.gpsimd.indirect_dma_start(
                            out=exch_ad[:], out_offset=bass.IndirectOffsetOnAxis(
                                ap=wn_ap, axis=0),
                            in_=astage[:], in_offset=None)
                    else:
                        h3stage = stg.tile([128, F3 + 16], DT_F32, tag="h3s")
                        nc.vector.tensor_copy(out=h3stage[:, :F3], in_=ph[:])
                        nc.vector.tensor_copy(out=h3stage[:, F3:], in_=pa[:])
                        nc.gpsimd.indirect_dma_start(
                            out=exch3[:], out_offset=bass.IndirectOffsetOnAxis(
                                ap=wn_ap, axis=0),
                            in_=h3stage[:], in_offset=None)

            edge_phase(1, Hf1, ALD1)
            rg = [list(range(NC))]
            nc.gpsimd.collective_compute("AllGather", ALU.bypass, replica_groups=rg,
                                         ins=[exch_h[0:SHARD, :].opt()], outs=[Hf2[:].opt()])
            edge_phase(2, Hf2, exch_ad)
            nc.gpsimd.collective_compute("AllGather", ALU.bypass, replica_groups=rg,
                                         ins=[exch3[0:SHARD, :].opt()], outs=[H3f[:].opt()])
            edge_phase(3, H3f, None, Hsrc_local=exch3)
            nc.sync.dma_start(out=OUT[:], in_=OUTI[0:SHARD, :])
            tk = sb.tile([128, 1], DT_F32, tag="tick")
            nc.sync.dma_start(out=tk[:], in_=P["tick"][:])
            nc.sync.dma_start(out=TOCK[:], in_=tk[:])

    _split_drain_waits(nc)
    return nc


_CACHE = {}
_last_in_maps = None


def kernel(**inputs):
    x = np.asarray(inputs["x"], np.float32)
    edge_index = np.asarray(inputs["edge_index"], np.int32)
    esrc, edst, dstrow, wnode, wnodeg, W, T = host_prep(edge_index)
    key = (W, T)
    if key not in _CACHE:
        _CACHE[key] = build_program(W, T)
    nc = _CACHE[key]

    com = {
        "xT": np.ascontiguousarray(x.T),
        "W1": np.asarray(inputs["W1"], np.float32),
        "W1Tc": chunk_rows(np.ascontiguousarray(np.asarray(inputs["W1"]).T)),
        "Wa1": chunk_rows(np.concatenate(
            [blockdiag(np.asarray(inputs["as1"])), blockdiag(np.asarray(inputs["ad1"]))], 1)),
        "W2c": chunk_rows(np.asarray(inputs["W2"], np.float32)),
        "W2Tc": chunk_rows(np.ascontiguousarray(np.asarray(inputs["W2"]).T)),
        "Wa2": chunk_rows(np.concatenate(
            [blockdiag(np.asarray(inputs["as2"])), blockdiag(np.asarray(inputs["ad2"]))], 1)),
        "W3c": chunk_rows(np.asarray(inputs["W3"], np.float32)),
        "W3T": np.ascontiguousarray(np.asarray(inputs["W3"]).T),
        "Wa3": np.concatenate(
            [blockdiag(np.asarray(inputs["as3"])), blockdiag(np.asarray(inputs["ad3"]))], 1),
        "b1t": np.tile(np.asarray(inputs["b1"], np.float32)[None, :], (128, 1)),
        "b2t": np.tile(np.asarray(inputs["b2"], np.float32)[None, :], (128, 1)),
        "b3t": np.tile(np.asarray(inputs["b3"], np.float32)[None, :], (128, 1)),
    }
    in_maps = []
    for c in range(NC):
        m = dict(com)
        m["esrc"] = esrc[c]
        m["edst"] = edst[c]
        m["dstrow"] = dstrow[c]
        m["wnode"] = wnode[c]
        m["wnodeg"] = wnodeg[c]
        m["tick"] = np.zeros((128, 1), np.float32)
        in_maps.append(m)
    global _last_in_maps
    _last_in_maps = in_maps
    res = run_bass_kernel_spmd(nc, in_maps, list(range(NC)))
    return np.concatenate([res.results[c]["out"] for c in range(NC)], axis=0)
